# revision 19
# baseline (speedup 1.0000x reference)
"""Dot-product stereo cost volume on 8 Trainium2 NeuronCores.

cost[b, d, y, x] = sum_c left[b,c,y,x] * right[b,c,y,x-d], zeros where x-d < 0.
Shapes: left/right [4, 128, 192, 640] fp32, D = 96 -> out [4, 96, 192, 640] fp32.

Strategy
--------
Sharding: 8 cores <- (b, y-half): core k handles batch k//2, rows 96*(k%2)..+96.
No halo needed (disparity shifts are along W only).

Per (y) row the math is a banded Gram matrix: G_y[x', x] = sum_c R[c,x'] L[c,x],
and cost[d, y, x] = G_y[x-d, x].  The PE computes G in M-row tiles:
tile t covers x' in [M*t, M*t+M), x in [M*t, M*t+M+96) (since d <= 95, every
needed (x', x) pair with x' in that M-block satisfies 0 <= x - x' <= M+95).
128//M such tiles stack into one [128, M+96] PSUM tile via tile_position column
groups.  The raw rect tiles stream to a DRAM scratch buffer in float16 (the
quantization error is relative to each stored value, ~2^-12, far inside the
2e-2 gate); the diagonal reindex (d = x - x') is absorbed into the host-side
unshard with one precomputed fancy index (a diagonal of G is not expressible
as a DMA access pattern: SBUF-side APs cannot couple partition and byte
offsets, and burst contiguity runs along d on the source but along x in the
output layout).

Engine budget per core (cost-model units, 2.4 GHz, 360 B/ns aggregate DMA):
loads 62.9 MB fp32 = 175 us; fp16 scratch at M=64 is 19.7 MB -> DMA 229 us
total.  fp32 matmul streams at 4 cyc/row, so PE = 96*(W/M)*(M+96)*4*0.417 ns
= 256 us at M=64 -- PE-bound.  mode="split3" instead feeds the PE fp16
hi/lo input planes (split on host; same 4 B/elem load traffic) and computes
G = r_hi l_hi + r_hi l_lo + r_lo l_hi in three 1-cyc/row passes accumulated
in PSUM: PE = 192 us, fp32-grade precision (lo*lo term ~2^-22 relative),
leaving the kernel DMA-bound at ~229 us.  mode="m128" keeps fp32 matmuls but
halves the per-row rhs restreaming (PE 179 us) at the cost of wider rect
junk (27.5 MB scratch -> DMA 251 us) -- the zero-precision-risk fallback.
"""

import sys

if "/opt/trn_rl_repo" not in sys.path:
    sys.path.insert(0, "/opt/trn_rl_repo")

import numpy as np

B, C, H, W = 4, 128, 192, 640
D = 96
HSH = H // 2          # rows per core

MODE = "t32"          # "base" (M=64 fp32 PE) | "m128" | "split3" | "t32"


def _geom(mode):
    # t64 computes with M=128 matmuls but stores the baseline M=64 layout
    # (the band shift happens during PSUM evacuation), so its host-visible
    # geometry is the M=64 one.
    mt = 128 if mode in ("m128", "t32") else 64   # M (x') tile height
    nw = mt + 96                         # free (x) tile width per matmul
    nt = W // mt                         # x'-tiles per row
    st = 128 // mt                       # tiles stacked per psum tile
    nps = nt // st                       # psum tiles per row
    return mt, nw, nt, st, nps


MT, NW, NT, ST, NPS = _geom(MODE)

_compiled = None

# consumed by bench.py only (not by the grading harness)
if MODE == "split3":
    BENCH_INPUTS = [(n, (C, HSH, W), "float16")
                    for n in ("left_hi", "left_lo", "right_hi", "right_lo")]
else:
    BENCH_INPUTS = [("left", (C, HSH, W), "float32"),
                    ("right", (C, HSH, W), "float32")]


def _build(repeat=1, mode=MODE, yb=2, lbufs=4, sbufs=4, pbufs=6, engsel=3,
           ldq="scalar", stq="sync"):
    import contextlib
    import concourse.bacc as bacc
    import concourse.tile as tile
    import concourse.mybir as mybir

    mt, nw, nt, st_, nps = _geom(mode)
    split = mode == "split3"
    t32 = mode == "t32"
    t64 = mode == "t64"
    if t64:
        # PE runs M=128 tiles; evacuation splits each [128, 224] psum tile
        # into two 64-row bands shifted by 64 cols, reproducing the M=64
        # rect layout (stores 160 cols/band instead of 224).
        mt, nw, nt, st_, nps = 128, 224, 5, 1, 5

    nc = bacc.Bacc("TRN2", target_bir_lowering=False, debug=False, num_devices=8)
    f32 = mybir.dt.float32
    f16 = mybir.dt.float16
    in_dt = f16 if split else f32

    if split:
        in_aps = {
            n: nc.dram_tensor(n, [C, HSH, W], f16, kind="ExternalInput").ap()
            for n in ("left_hi", "left_lo", "right_hi", "right_lo")
        }
    else:
        in_aps = {
            n: nc.dram_tensor(n, [C, HSH, W], f32, kind="ExternalInput").ap()
            for n in ("left", "right")
        }
    # y-pair-major, p-major layout: one store covers 2 rows as a single
    # plain contiguous-per-partition DMA; float16 payload.  In t32 mode the
    # per-y row is [5 blocks x 128 cols] of 32-row bands (see below) instead
    # of [NPS x NW] rect tiles.
    rw_scr = 5 * 128 if t32 else (5 * 160 if t64 else nps * nw)
    scr_ap = nc.dram_tensor(
        "scr", [HSH // 2, 128, 2 * rw_scr], f16, kind="ExternalOutput"
    ).ap()

    WPAD = W + 96  # L is zero-padded on the right so every rhs window is full
    YB = yb        # y rows loaded per input DMA

    lplanes = ["left_hi", "left_lo"] if split else ["left"]
    rplanes = ["right_hi", "right_lo"] if split else ["right"]

    with tile.TileContext(nc) as tc:
        with (
            tc.tile_pool(name="lpool", bufs=lbufs) as lpool,
            tc.tile_pool(name="rpool", bufs=lbufs) as rpool,
            tc.tile_pool(name="stage", bufs=sbufs) as stage_pool,
            tc.tile_pool(name="psum", bufs=pbufs, space="PSUM") as psum_pool,
        ):
            rep_ctx = (
                tc.For_i(0, repeat, 1) if repeat > 1 else contextlib.nullcontext()
            )
            with rep_ctx:
                for y0 in range(0, HSH, YB):
                    # [c, (y pair, x)] input tiles; loads on the ACT HWDGE
                    # ring so they round-robin against stores on the SP ring
                    ldeng = getattr(nc, ldq)
                    lts, rts = {}, {}
                    for n in lplanes:
                        lt = lpool.tile([128, YB * WPAD], in_dt, name=f"lt_{n}_{y0}",
                                        tag=f"lt_{n}")
                        lt3 = lt.rearrange("c (y w) -> c y w", y=YB)
                        ldeng.dma_start(lt3[:, :, 0:W], in_aps[n][:, y0:y0 + YB, :])
                        nc.vector.memset(lt3[:, :, W:WPAD], 0.0)
                        lts[n] = lt
                    for n in rplanes:
                        rt = rpool.tile([128, YB * W], in_dt, name=f"rt_{n}_{y0}",
                                        tag=f"rt_{n}")
                        ldeng.dma_start(
                            rt.rearrange("c (y w) -> c y w", y=YB),
                            in_aps[n][:, y0:y0 + YB, :],
                        )
                        rts[n] = rt

                    if split:
                        # G = r_hi l_hi + r_hi l_lo + r_lo l_hi (lo*lo dropped,
                        # ~2^-22 relative) accumulated in fp32 PSUM
                        passes = [
                            (rts["right_hi"], lts["left_hi"]),
                            (rts["right_hi"], lts["left_lo"]),
                            (rts["right_lo"], lts["left_hi"]),
                        ]
                    else:
                        passes = [(rts["right"], lts["left"])]

                    RW = rw_scr    # per-row stage width
                    stg = stage_pool.tile([128, 2 * RW], f16, name=f"st_{y0}", tag="st")
                    for yi in range(YB):
                        for s in range(nps):
                            ps = psum_pool.tile([128, nw], f32,
                                                name=f"ps_{y0 + yi}_{s}", tag="ps")
                            for u in range(st_):
                                t = st_ * s + u
                                q0 = yi * WPAD + mt * t
                                for pi, (rt, lt) in enumerate(passes):
                                    nc.tensor.matmul(
                                        ps[mt * u: mt * (u + 1), :],
                                        lhsT=rt[:, yi * W + mt * t: yi * W + mt * t + mt],
                                        rhs=lt[:, q0: q0 + nw],
                                        start=(pi == 0),
                                        stop=(pi == len(passes) - 1),
                                        tile_position=None if st_ == 1 else (0, mt * u),
                                    )
                            if t64:
                                # two 64-row bands, cols [64g, 64g+160), into
                                # the M=64-rect stage layout; DVE takes g=0,
                                # GPSIMD g=1, keeping ACT free to issue loads
                                blk = (yi * 5 + s) * 160
                                for g in range(2):
                                    src = ps[64 * g: 64 * g + 64,
                                             64 * g: 64 * g + 160]
                                    dst = stg[64 * g: 64 * g + 64,
                                              blk: blk + 160]
                                    if g == 0:
                                        nc.vector.tensor_copy(dst, src)
                                    else:
                                        nc.gpsimd.tensor_copy(dst, src)
                            elif t32:
                                # Tight-band evacuation: the [128, 224] rect
                                # tile holds, for partition p (x' = 128s+p),
                                # useful cols f in [p, p+96).  Store only the
                                # 32-row band windows [32g, 32g+128) -- junk
                                # 25% instead of rect's 43% -- with the
                                # per-band shift folded into three otherwise
                                # idle engines' copies (a full per-partition
                                # shear is not expressible in any single op).
                                blk = (yi * 5 + s) * 128
                                for g in range(4):
                                    src = ps[32 * g: 32 * g + 32,
                                             32 * g: 32 * g + 128]
                                    dst = stg[32 * g: 32 * g + 32,
                                              blk: blk + 128]
                                    eng = (s + g) % engsel
                                    if eng == 0:
                                        nc.vector.tensor_copy(dst, src)
                                    elif eng == 1:
                                        nc.scalar.copy(dst, src)
                                    else:
                                        nc.gpsimd.tensor_copy(dst, src)
                            else:
                                nc.vector.tensor_copy(
                                    stg[:, yi * RW + s * nw: yi * RW + (s + 1) * nw],
                                    ps[:],
                                )
                    # one plain contiguous store per y-pair; the right-edge
                    # junk of the last psum tile block rides along (host
                    # never reads it) -- keeping the AP trivial
                    getattr(nc, stq).dma_start(scr_ap[y0 // 2], stg[:])

    nc.compile()
    return nc


def _host_index():
    """idx[d, x] -> flat offset into scr[y] holding G[x-d, x].

    Valid only where x >= d; mask handles the rest.
    """
    d = np.arange(D)[:, None]
    x = np.arange(W)[None, :]
    xp = np.maximum(x - d, 0)        # x' = x - d
    if MODE == "t32":
        # scr[y] is [128 p, 5 s, 128 j]: band (s, g) holds x' = 128s + 32g + q
        # (partition p = 32g + q) at j = x - 128s - 32g; j = d + (x'%32) < 128
        s = xp // 128
        p = xp - 128 * s
        g = p // 32
        j = x - 128 * s - 32 * g
        idx = (p * 5 + s) * 128 + j
    else:
        t = xp // MT                 # x'-tile
        q = xp - MT * t              # row within tile
        s = t // ST                  # psum tile
        u = t - ST * s               # col group within psum tile
        f = x - MT * t               # col within tile (< NW always)
        p = MT * u + q               # psum partition
        idx = (p * NPS + s) * NW + f  # scr[y] is [128 p, NPS s, NW f]
    mask = (x >= d)
    return idx.astype(np.int64), mask


def kernel(left, right, num_disparities):
    global _compiled
    left = np.asarray(left)
    right = np.asarray(right)
    assert int(num_disparities) == D
    assert left.shape == (B, C, H, W) and right.shape == (B, C, H, W)

    if _compiled is None:
        _compiled = _build()
    nc = _compiled

    from concourse.bass_utils import run_bass_kernel_spmd

    split = MODE == "split3"
    if split:
        lh = left.astype(np.float16)
        ll = (left - lh.astype(np.float32)).astype(np.float16)
        rh = right.astype(np.float16)
        rl = (right - rh.astype(np.float32)).astype(np.float16)

    in_maps = []
    for k in range(8):
        b, hh = k // 2, k % 2
        sl = slice(96 * hh, 96 * hh + 96)
        if split:
            in_maps.append({
                "left_hi": np.ascontiguousarray(lh[b, :, sl, :]),
                "left_lo": np.ascontiguousarray(ll[b, :, sl, :]),
                "right_hi": np.ascontiguousarray(rh[b, :, sl, :]),
                "right_lo": np.ascontiguousarray(rl[b, :, sl, :]),
            })
        else:
            in_maps.append({
                "left": np.ascontiguousarray(left[b, :, sl, :]),
                "right": np.ascontiguousarray(right[b, :, sl, :]),
            })

    res = run_bass_kernel_spmd(nc, in_maps, list(range(8)))

    idx, mask = _host_index()
    out = np.zeros((B, D, H, W), dtype=np.float32)
    for k in range(8):
        b, hh = k // 2, k % 2
        # scr is [48 y-pairs, 128 p, 2*rw] f16; un-pair to [96, 128*rw]
        rw = 5 * 128 if MODE == "t32" else NPS * NW
        scr = (
            res.results[k]["scr"]
            .reshape(HSH // 2, 128, 2, rw)
            .swapaxes(1, 2)
            .reshape(HSH, 128 * rw)
        )
        gathered = scr[:, idx.ravel()].astype(np.float32).reshape(HSH, D, W)
        gathered *= mask[None, :, :]
        out[b, :, 96 * hh: 96 * hh + 96, :] = gathered.transpose(1, 0, 2)
    return out
